# revision 22
# baseline (speedup 1.0000x reference)
"""Trainium2 Bass kernel for pointer-generator final-distribution (scatter_memory).

out[r, v] = p_gens[r] * vocab_ds[r, v]  (+ (1-p_gens[r])*attns[r, l_win]  at
v == sources[l, b(r)], duplicate source ids resolved last-occurrence-wins)

Strategy (8 NeuronCores, SPMD), DMA-roofline bound (~360 GB/s/core across
the 16 shared DMA engines):
  - Shard by batch column: core k owns b in {4k..4k+3}, all T decoder steps
    (rows r = t*B + b). Host pre-gathers rows b-major so device DMAs are
    contiguous; two 128-row groups per core (2 b's x 64 t each), streams
    interleaved window-by-window.
  - All heavy HBM traffic is bf16 (host converts in, upcasts out) — halves
    bytes vs f32; max rel err ~1e-2 vs the 2e-2 gate. Vocab loads are padded
    row-major per group; output stores are window-major (each [128, 4096]
    block contiguous) for linear HBM write sweeps.
  - The scatter: updates are re-indexed per 2048-wide "quad"; the device
    builds the one-hot rhs itself with a single fp16 is_equal(iota, ck) per
    quad (alternating DVE/Pool; integers <= 2047 are fp16-exact so this is
    bit-identical to a host one-hot at ~0 DMA cost), PE does one
    [Kq,128]x[Kq,2048] bf16 matmul per quad into f32 PSUM, and one wide DVE
    add per quad folds it into the gated tile (wide ops amortize the fixed
    ~200-cycle PSUM access bubble).
  - ACT applies the p gating into a fresh out tile; loads dispatch from
    sync (g0) / gpsimd (g1), stores + prelude from scalar — no engine has
    a store wait blocking later load dispatches, and early transfer
    completion semaphores spread across all three queues.
"""

import numpy as np

N_CORES = 8
WIN = 4096
SUB = 512
QW = 2048   # quad width (columns per one-hot matmul)
KQH = 64    # vals/ck DRAM rows reserved per quad


def _host_prep(vocab_ds, attns, p_gens, sources, T):
    import ml_dtypes
    bf16 = ml_dtypes.bfloat16
    fp16 = np.float16
    f32 = np.float32
    vocab_ds = np.ascontiguousarray(vocab_ds, dtype=f32)
    attns = np.ascontiguousarray(attns, dtype=f32)
    p_gens = np.ascontiguousarray(p_gens, dtype=f32)
    src = np.asarray(sources).astype(np.int64)
    rows, V = vocab_ds.shape
    L, B = src.shape
    assert rows == T * B

    ag = (f32(1.0) - p_gens) * attns  # gated copy dist, [rows, L]

    # winners per batch column: duplicate source ids -> last occurrence wins
    wins = []
    for b in range(B):
        d = {}
        col = src[:, b]
        for l in range(L):
            d[int(col[l])] = l
        cols = np.fromiter(d.keys(), dtype=np.int64)
        ls = np.fromiter(d.values(), dtype=np.int64)
        o = np.argsort(cols)
        wins.append((cols[o], ls[o]))

    NW = (V + WIN - 1) // WIN
    # quad geometry: quads tile each window; (w, q, c0_abs, width)
    quad_geom = []
    for w in range(NW):
        ww = min(WIN, V - w * WIN)
        for q in range((ww + QW - 1) // QW):
            quad_geom.append((w, q, w * WIN + q * QW, min(QW, ww - q * QW)))
    NQ = len(quad_geom)
    quad_of = {}
    for j, (w, q, c0, wd) in enumerate(quad_geom):
        quad_of[(w, q)] = j
    QPW = WIN // QW  # quads per full window

    BPC = B // N_CORES  # 4
    G = BPC // 2        # 2 groups of 2 b's

    # bucket updates per (core, g, quad)
    upd = [[[[] for _ in range(NQ)] for _ in range(G)] for _ in range(N_CORES)]
    for core in range(N_CORES):
        for g in range(G):
            for half in range(2):
                b = core * BPC + g * 2 + half
                cols, ls = wins[b]
                for c, l in zip(cols.tolist(), ls.tolist()):
                    w = c // WIN
                    q = (c - w * WIN) // QW
                    upd[core][g][quad_of[(w, q)]].append((half, c, l))

    # uniform-per-(g, quad) Kq across cores (one NEFF shared SPMD)
    K_qs = [[max(len(upd[core][g][j]) for core in range(N_CORES)) for j in range(NQ)]
            for g in range(G)]
    assert all(k <= KQH for g in range(G) for k in K_qs[g]), \
        "quad update count exceeds the reserved row budget"
    # per-(g, window): first quad index, #quads, max Kq
    win_info = []
    for g in range(G):
        wi = []
        for w in range(NW):
            idxs = [j for j, (w2, _, _, _) in enumerate(quad_geom) if w2 == w]
            j0, nq = idxs[0], len(idxs)
            kw = max(K_qs[g][j] for j in idxs)
            wi.append((j0, nq, kw))
        win_info.append(wi)

    # per-core device inputs
    in_maps = []
    iota = np.broadcast_to(np.arange(QW, dtype=fp16), (KQH, QW)).copy()
    for core in range(N_CORES):
        m = {"iota": iota}
        for g in range(G):
            row_idx = []
            for half in range(2):
                b = core * BPC + g * 2 + half
                row_idx.extend(t * B + b for t in range(T))
            row_idx = np.asarray(row_idx)
            vg = vocab_ds[row_idx].astype(bf16)
            vb = np.zeros((128, NW * WIN), dtype=bf16)
            vb[:, :V] = vg
            m[f"vocab{g}"] = vb
            m[f"pgen{g}"] = p_gens[row_idx]
            # per-window metadata: vals blocks [KQH, QPW*128] bf16 (quad q
            # of window w at rows w*KQH+k, cols q*128..); ck [KQH, NQ] fp16
            vals = np.zeros((NW * KQH, QPW * 128), dtype=f32)
            ck = np.full((KQH, NQ), -1.0, dtype=f32)
            for j in range(NQ):
                w, q, c0, wd = quad_geom[j]
                for k, (half, c, l) in enumerate(upd[core][g][j]):
                    r0 = half * T
                    vals[w * KQH + k, q * 128 + r0: q * 128 + r0 + T] = \
                        ag[row_idx[r0: r0 + T], l]
                    ck[k, j] = f32(c - c0)
            m[f"vals{g}"] = vals.astype(bf16)
            m[f"ck{g}"] = ck
        in_maps.append(m)

    meta = dict(V=V, T=T, B=B, NW=NW, NQ=NQ, G=G, quad_geom=quad_geom,
                K_qs=K_qs, BPC=BPC, win_info=win_info, QPW=QPW)
    return in_maps, meta


def _build_nc(meta):
    from concourse import bacc, mybir

    V, NW, NQ, G = meta["V"], meta["NW"], meta["NQ"], meta["G"]
    quad_geom, K_qs, QPW = meta["quad_geom"], meta["K_qs"], meta["QPW"]
    f32 = mybir.dt.float32
    fp16 = mybir.dt.float16
    bf16 = mybir.dt.bfloat16

    nc = bacc.Bacc(None, target_bir_lowering=False, debug=False)
    vocab = [nc.declare_dram_parameter(f"vocab{g}", [128, NW * WIN], bf16,
                                       isOutput=False) for g in range(G)]
    pgen = [nc.declare_dram_parameter(f"pgen{g}", [128, 1], f32, isOutput=False)
            for g in range(G)]
    vals = [nc.declare_dram_parameter(f"vals{g}", [NW * KQH, QPW * 128], bf16,
                                      isOutput=False) for g in range(G)]
    ckp = [nc.declare_dram_parameter(f"ck{g}", [KQH, NQ], f32, isOutput=False)
           for g in range(G)]
    iota = nc.declare_dram_parameter("iota", [KQH, QW], fp16, isOutput=False)
    out = [nc.declare_dram_parameter(f"out{g}", [NW * 128, WIN], bf16, isOutput=True)
           for g in range(G)]

    from concourse.tile import TileContext

    win_info = meta["win_info"]
    with TileContext(nc) as tc:
        with tc.tile_pool(name="in", bufs=6) as in_pool, \
             tc.tile_pool(name="out", bufs=8) as out_pool, \
             tc.tile_pool(name="small", bufs=1) as small, \
             tc.tile_pool(name="vals", bufs=4) as vals_pool, \
             tc.tile_pool(name="ohq", bufs=4) as ohq_pool, \
             tc.tile_pool(name="psum", bufs=2, space="PSUM") as psum_pool:

            ldq = [nc.sync, nc.gpsimd]
            iota_t = small.tile([KQH, QW], fp16, tag="iota")
            nc.scalar.dma_start(out=iota_t[:], in_=iota[:])
            p_t, ck_t = [], []
            for g in range(G):
                pt = small.tile([128, 1], f32, tag=f"p{g}")
                nc.scalar.dma_start(out=pt[:], in_=pgen[g][:])
                p_t.append(pt)
                ct = small.tile([KQH, NQ], f32, tag=f"ck{g}")
                nc.scalar.dma_start(out=ct[:], in_=ckp[g][:])
                ck_t.append(ct)

            oddq = 0
            for w in range(NW):
                c0w = w * WIN
                ww = min(WIN, V - c0w)
                for g in range(G):
                    j0, nq, kw = win_info[g][w]
                    t_in = in_pool.tile([128, WIN], bf16, tag="in")
                    ldq[g].dma_start(out=t_in[:, :ww],
                                     in_=vocab[g][:, c0w:c0w + ww])
                    vals_t = vals_pool.tile([KQH, QPW * 128], bf16, tag="v")
                    if kw > 0:
                        ldq[g].dma_start(out=vals_t[:kw, :],
                                         in_=vals[g][w * KQH:w * KQH + kw, :])
                    t = out_pool.tile([128, WIN], bf16, tag="out")
                    nc.scalar.activation(
                        t[:, :ww], t_in[:, :ww],
                        mybir.ActivationFunctionType.Copy, scale=p_t[g][:, :1])
                    for jq in range(nq):
                        j = j0 + jq
                        Kq = K_qs[g][j]
                        if Kq == 0:
                            continue
                        _, _, c0, wd = quad_geom[j]
                        lo = c0 - c0w
                        # device-built one-hot: fp16 iota==ck, bf16 out
                        oh_q = ohq_pool.tile([KQH, QW], bf16, tag="oh")
                        eng = nc.vector if oddq % 2 == 0 else nc.gpsimd
                        oddq += 1
                        eng.tensor_scalar(
                            out=oh_q[:Kq, :wd], in0=iota_t[:Kq, :wd],
                            scalar1=ck_t[g][:Kq, j:j + 1], scalar2=None,
                            op0=mybir.AluOpType.is_equal)
                        ps = psum_pool.tile([128, QW], f32, tag="ps")
                        for so in range(0, wd, SUB):
                            sw = min(SUB, wd - so)
                            nc.tensor.matmul(
                                out=ps[:, so:so + sw],
                                lhsT=vals_t[:Kq, jq * 128:(jq + 1) * 128],
                                rhs=oh_q[:Kq, so:so + sw],
                                start=True, stop=True)
                        nc.vector.tensor_add(
                            out=t[:, lo:lo + wd], in0=t[:, lo:lo + wd],
                            in1=ps[:, :wd])
                    nc.scalar.dma_start(out=out[g][w * 128:w * 128 + 128, :ww],
                                        in_=t[:, :ww])
    nc.finalize()
    return nc


def _unshard(res, meta, rows, V):
    B, BPC, G, T, NW = meta["B"], meta["BPC"], meta["G"], meta["T"], meta["NW"]
    full = np.empty((rows, V), dtype=np.float32)
    for core in range(N_CORES):
        for g in range(G):
            blk = np.asarray(res.results[core][f"out{g}"], dtype=np.float32)
            flat = np.empty((128, V), dtype=np.float32)
            for w in range(NW):
                c0w = w * WIN
                ww = min(WIN, V - c0w)
                flat[:, c0w:c0w + ww] = blk[w * 128:w * 128 + 128, :ww]
            for half in range(2):
                b = core * BPC + g * 2 + half
                full[b::B] = flat[half * T:(half + 1) * T]
    return full


def kernel(vocab_ds, attns, p_gens, sources, decoder_batch_len):
    T = int(decoder_batch_len)
    in_maps, meta = _host_prep(vocab_ds, attns, p_gens, sources, T)
    nc = _build_nc(meta)

    from concourse.bass_utils import run_bass_kernel_spmd
    res = run_bass_kernel_spmd(nc, in_maps, list(range(N_CORES)))

    rows, V = np.asarray(vocab_ds).shape
    return _unshard(res, meta, rows, V)


# revision 23
# speedup vs baseline: 4.5959x; 4.5959x over previous
"""Trainium2 Bass kernel for pointer-generator final-distribution (scatter_memory).

out[r, v] = p_gens[r] * vocab_ds[r, v]  (+ (1-p_gens[r])*attns[r, l_win]  at
v == sources[l, b(r)], duplicate source ids resolved last-occurrence-wins)

Strategy (8 NeuronCores, SPMD), DMA-roofline bound (~360 GB/s/core across
the 16 shared DMA engines):
  - Shard by batch column: core k owns b in {4k..4k+3}, all T decoder steps
    (rows r = t*B + b). Host pre-gathers rows b-major so device DMAs are
    contiguous; two 128-row groups per core (2 b's x 64 t each). The two
    group streams are interleaved window-by-window so neither drains while
    the other warms up.
  - All heavy HBM traffic is bf16 (host converts in, upcasts out) — halves
    bytes vs f32; max rel err ~1e-2 vs the 2e-2 gate. DRAM buffers are laid
    out window-major (each [128, 8192] window block contiguous) so every
    DMA is a linear ~2MB HBM sweep of full-width 16KB descriptors.
  - The scatter is a compact one-hot matmul on the otherwise-idle PE: per
    512-wide subtile the host bakes [K, 128] bf16 update values
    (block-diagonal over the two b's) and the matching one-hot [K, 512]
    rhs (precomputed host-side — cheaper than a 512-cycle DVE is_equal per
    subtile); PE computes proj = vals.T @ onehot into f32 PSUM.
  - ACT applies the p gating into a fresh out tile (keeps the act/add/store
    chain out-of-place so engines decouple across windows); the PSUM adds
    alternate DVE <-> Pool per subtile. Loads dispatch from sync, stores
    from Pool — no engine has a store wait blocking later load dispatches.
"""

import numpy as np

N_CORES = 8
WIN = 4096
SUB = 512
KH = 32  # one-hot DRAM rows reserved per window block


def _host_prep(vocab_ds, attns, p_gens, sources, T):
    import ml_dtypes
    bf16 = ml_dtypes.bfloat16
    f32 = np.float32
    vocab_ds = np.ascontiguousarray(vocab_ds, dtype=f32)
    attns = np.ascontiguousarray(attns, dtype=f32)
    p_gens = np.ascontiguousarray(p_gens, dtype=f32)
    src = np.asarray(sources).astype(np.int64)
    rows, V = vocab_ds.shape
    L, B = src.shape
    assert rows == T * B

    ag = (f32(1.0) - p_gens) * attns  # gated copy dist, [rows, L]

    # winners per batch column: duplicate source ids -> last occurrence wins
    wins = []
    for b in range(B):
        d = {}
        col = src[:, b]
        for l in range(L):
            d[int(col[l])] = l
        cols = np.fromiter(d.keys(), dtype=np.int64)
        ls = np.fromiter(d.values(), dtype=np.int64)
        o = np.argsort(cols)
        wins.append((cols[o], ls[o]))

    NW = (V + WIN - 1) // WIN
    # subtile geometry, shared by all cores/groups
    sub_geom = []  # (w, s, c0_abs, width)
    for w in range(NW):
        ww = min(WIN, V - w * WIN)
        for s in range((ww + SUB - 1) // SUB):
            sub_geom.append((w, s, w * WIN + s * SUB, min(SUB, ww - s * SUB)))
    NS = len(sub_geom)
    sub_of = {}
    for i, (w, s, c0, wd) in enumerate(sub_geom):
        sub_of[(w, s)] = i

    BPC = B // N_CORES  # 4
    G = BPC // 2        # 2 groups of 2 b's

    # bucket updates per (core, g, subtile)
    upd = [[[[] for _ in range(NS)] for _ in range(G)] for _ in range(N_CORES)]
    for core in range(N_CORES):
        for g in range(G):
            for half in range(2):
                b = core * BPC + g * 2 + half
                cols, ls = wins[b]
                for c, l in zip(cols.tolist(), ls.tolist()):
                    w = c // WIN
                    s = (c - w * WIN) // SUB
                    i = sub_of[(w, s)]
                    upd[core][g][i].append((half, c, l))

    # uniform-per-(g, subtile) K across cores (one NEFF shared SPMD)
    K_ws = [[max(len(upd[core][g][i]) for core in range(N_CORES)) for i in range(NS)]
            for g in range(G)]
    assert all(k <= 128 for g in range(G) for k in K_ws[g]), \
        "subtile update count exceeds the 128-partition budget"
    # per-(g, window): first subtile index, #subtiles, max K (partition
    # extent of the one-hot load for that window)
    win_info = []
    for g in range(G):
        wi = []
        for w in range(NW):
            idxs = [i for i, (w2, s2, _, _) in enumerate(sub_geom) if w2 == w]
            i0, nsub = idxs[0], len(idxs)
            kw = max(K_ws[g][i] for i in idxs)
            assert kw <= KH
            wi.append((i0, nsub, kw))
        win_info.append(wi)
    max_nsub = max(wi[1] for g in range(G) for wi in win_info[g])

    # per-core device inputs (window-major blocked layouts)
    in_maps = []
    for core in range(N_CORES):
        m = {}
        for g in range(G):
            row_idx = []
            for half in range(2):
                b = core * BPC + g * 2 + half
                row_idx.extend(t * B + b for t in range(T))
            row_idx = np.asarray(row_idx)
            vg = vocab_ds[row_idx].astype(bf16)
            vb = np.zeros((NW * 128, WIN), dtype=bf16)
            for w in range(NW):
                c0w = w * WIN
                ww = min(WIN, V - c0w)
                vb[w * 128:w * 128 + 128, :ww] = vg[:, c0w:c0w + ww]
            m[f"vocab{g}"] = vb
            m[f"pgen{g}"] = p_gens[row_idx]
            # per-window metadata block [KH, max_nsub*(SUB+128)] bf16:
            # cols [0, nsub*SUB) one-hot rhs, cols [max_nsub*SUB, +nsub*128)
            # the matmul lhsT update values — one DMA per window loads both
            vstride = max_nsub * SUB
            oh = np.zeros((NW * KH, max_nsub * (SUB + 128)), dtype=bf16)
            vals = np.zeros((128, NS * 128), dtype=f32)
            for i in range(NS):
                w, s, c0, wd = sub_geom[i]
                for k, (half, c, l) in enumerate(upd[core][g][i]):
                    # rows of this b occupy partitions half*T .. half*T+T
                    r0 = half * T
                    vals[k, i * 128 + r0: i * 128 + r0 + T] = ag[row_idx[r0: r0 + T], l]
                    oh[w * KH + k, s * SUB + (c - c0)] = 1.0
            valsb = vals.astype(bf16)
            for i in range(NS):
                w, s, _, _ = sub_geom[i]
                oh[w * KH:(w + 1) * KH, vstride + s * 128:vstride + (s + 1) * 128] = \
                    valsb[:KH, i * 128:(i + 1) * 128]
            m[f"oh{g}"] = oh
        in_maps.append(m)

    meta = dict(V=V, T=T, B=B, NW=NW, NS=NS, G=G, sub_geom=sub_geom,
                sub_of=sub_of, K_ws=K_ws, BPC=BPC, win_info=win_info,
                max_nsub=max_nsub)
    return in_maps, meta


def _build_nc(meta):
    from concourse import bacc, mybir

    V, NW, NS, G = meta["V"], meta["NW"], meta["NS"], meta["G"]
    sub_geom, K_ws = meta["sub_geom"], meta["K_ws"]
    max_nsub = meta["max_nsub"]
    f32 = mybir.dt.float32

    bf16 = mybir.dt.bfloat16
    nc = bacc.Bacc(None, target_bir_lowering=False, debug=False)
    vocab = [nc.declare_dram_parameter(f"vocab{g}", [NW * 128, WIN], bf16,
                                       isOutput=False) for g in range(G)]
    pgen = [nc.declare_dram_parameter(f"pgen{g}", [128, 1], f32, isOutput=False)
            for g in range(G)]
    ohp = [nc.declare_dram_parameter(f"oh{g}", [NW * KH, max_nsub * (SUB + 128)],
                                     bf16, isOutput=False) for g in range(G)]
    out = [nc.declare_dram_parameter(f"out{g}", [NW * 128, WIN], bf16, isOutput=True)
           for g in range(G)]

    from concourse.tile import TileContext

    win_info = meta["win_info"]
    with TileContext(nc) as tc:
        METAW = max_nsub * (SUB + 128)
        vstride = max_nsub * SUB
        with tc.tile_pool(name="in", bufs=6) as in_pool, \
             tc.tile_pool(name="out", bufs=10) as out_pool, \
             tc.tile_pool(name="small", bufs=1) as small, \
             tc.tile_pool(name="oh", bufs=4) as oh_pool, \
             tc.tile_pool(name="psum", bufs=2, space="PSUM") as psum_pool:

            # load queues: g0 -> sync, g1 -> gpsimd; prelude + stores ride
            # the scalar queue (empty early / late respectively) so early
            # transfer-completion semaphores don't serialize on one queue.
            ldq = [nc.sync, nc.gpsimd]
            p_t = []
            for g in range(G):
                pt = small.tile([128, 1], f32, tag=f"p{g}")
                nc.scalar.dma_start(out=pt[:], in_=pgen[g][:])
                p_t.append(pt)

            for w in range(NW):
                c0w = w * WIN
                ww = min(WIN, V - c0w)
                for g in range(G):
                    i0, nsub, kw = win_info[g][w]
                    t_in = in_pool.tile([128, WIN], bf16, tag="in")
                    ldq[g].dma_start(out=t_in[:, :ww],
                                     in_=vocab[g][w * 128:w * 128 + 128, :ww])
                    oh_t = oh_pool.tile([128, METAW], bf16, tag="oh")
                    if kw > 0:
                        ldq[g].dma_start(
                            out=oh_t[:kw, :],
                            in_=ohp[g][w * KH:w * KH + kw, :])
                    t = out_pool.tile([128, WIN], bf16, tag="out")
                    nc.scalar.activation(
                        t[:, :ww], t_in[:, :ww],
                        mybir.ActivationFunctionType.Copy, scale=p_t[g][:, :1])
                    # pairs of subtiles share one [128, 1024] PSUM tile so a
                    # single wide DVE add amortizes the fixed PSUM-access
                    # bubble (~200 cycles per DVE instruction)
                    for sp in range(0, nsub, 4):
                        pair = [s for s in range(sp, min(sp + 4, nsub))]
                        if all(K_ws[g][i0 + s] == 0 for s in pair):
                            continue
                        lo = sub_geom[i0 + sp][2] - c0w
                        wtot = sum(sub_geom[i0 + s][3] for s in pair)
                        ps = psum_pool.tile([128, 4 * SUB], f32, tag="ps")
                        for s in pair:
                            i = i0 + s
                            K = max(K_ws[g][i], 1)
                            _, _, c0, wd = sub_geom[i]
                            po = (s - sp) * SUB
                            nc.tensor.matmul(
                                out=ps[:, po:po + wd],
                                lhsT=oh_t[:K, vstride + s * 128:vstride + (s + 1) * 128],
                                rhs=oh_t[:K, s * SUB:s * SUB + wd],
                                start=True, stop=True)
                        nc.vector.tensor_add(
                            out=t[:, lo:lo + wtot], in0=t[:, lo:lo + wtot],
                            in1=ps[:, :wtot])
                    nc.scalar.dma_start(out=out[g][w * 128:w * 128 + 128, :ww],
                                        in_=t[:, :ww])
    nc.finalize()
    return nc


def _unshard(res, meta, rows, V):
    B, BPC, G, T, NW = meta["B"], meta["BPC"], meta["G"], meta["T"], meta["NW"]
    full = np.empty((rows, V), dtype=np.float32)
    for core in range(N_CORES):
        for g in range(G):
            blk = np.asarray(res.results[core][f"out{g}"], dtype=np.float32)
            flat = np.empty((128, V), dtype=np.float32)
            for w in range(NW):
                c0w = w * WIN
                ww = min(WIN, V - c0w)
                flat[:, c0w:c0w + ww] = blk[w * 128:w * 128 + 128, :ww]
            for half in range(2):
                b = core * BPC + g * 2 + half
                full[b::B] = flat[half * T:(half + 1) * T]
    return full


def kernel(vocab_ds, attns, p_gens, sources, decoder_batch_len):
    T = int(decoder_batch_len)
    in_maps, meta = _host_prep(vocab_ds, attns, p_gens, sources, T)
    nc = _build_nc(meta)

    from concourse.bass_utils import run_bass_kernel_spmd
    res = run_bass_kernel_spmd(nc, in_maps, list(range(N_CORES)))

    rows, V = np.asarray(vocab_ds).shape
    return _unshard(res, meta, rows, V)
